# revision 23
# baseline (speedup 1.0000x reference)
"""CovariantEvolutionBlock Trainium2 kernel.

Strategy: token-parallel over B*L across 8 cores (512 tokens/core), zero
collectives. Each core recomputes full-batch K/V for attention (inputs are
rotated per-core so "own" tokens are always columns 0:512; sigmoid attention
is permutation-invariant over keys). Activations are kept feature-major
[dims, tokens] on-chip so matmul chains need no transposes; weights are
pre-transposed/cast to bf16 on the host. All matmuls are bf16 with fp32 PSUM
accumulation.
"""

import sys

try:
    import concourse.bass as bass  # noqa: F401
except ImportError:
    sys.path.insert(0, "/opt/trn_rl_repo")

import numpy as np
import ml_dtypes

import concourse.bacc as bacc
import concourse.tile as tile
import concourse.mybir as mybir
from concourse.bass_utils import run_bass_kernel_spmd

F32 = mybir.dt.float32
BF16 = mybir.dt.bfloat16
AF = mybir.ActivationFunctionType

B, L, D, H, HD = 2, 2048, 1024, 16, 64
EPS = 1e-6
NCORES = 8
TOK = 512          # own tokens per core
KEYS = 2048        # keys per batch
KC = D // 128      # 8 feature chunks of 128
NTB = KEYS // TOK  # 4 token blocks per batch
OBW = 256          # out-block width (2 m-chunks) per psum tile


def _bias_ap(dram_ap):
    # [dim] -> [128, dim//128]: tile[p, c] = bias[c*128 + p]
    return dram_ap.rearrange("(c p) -> p c", p=128)


def build_program(dt_val: float, temp_val: float):
    nc = bacc.Bacc("TRN2", target_bir_lowering=False, debug=False,
                   num_devices=NCORES)

    d_in = {}
    for name, shape, dt in [
        ("zT", [D, KEYS], F32), ("cT", [D, KEYS], F32),
        ("fw1T", [D, 2 * D], BF16), ("fw2T", [2 * D, D], BF16),
        ("gw1T", [2 * D, D], BF16), ("gw2T", [D, D], BF16),
        ("qwT", [2 * D, D], BF16), ("kwT", [2 * D, D], BF16),
        ("vwT", [D, D], BF16), ("owT", [H, HD, D], BF16),
        ("cuw1T", [3 * D, 2 * D], BF16), ("cuw2T", [2 * D, D], BF16),
        ("mw1T", [D, 4 * D], BF16), ("mw2T", [4 * D, D], BF16),
        ("fb1", [2 * D], F32), ("fb2", [D], F32),
        ("gb1", [D], F32), ("gb2", [D], F32),
        ("cub1", [2 * D], F32), ("cub2", [D], F32),
        ("mb1", [4 * D], F32), ("mb2", [D], F32),
        ("wz", [D], F32), ("wc", [D], F32), ("wmlp", [D], F32),
    ]:
        d_in[name] = nc.dram_tensor(name, shape, dt, kind="ExternalInput").ap()

    z2T_d = nc.dram_tensor("z2T", [D, TOK], F32, kind="ExternalOutput").ap()
    connT_d = nc.dram_tensor("connT", [D, TOK], F32, kind="ExternalOutput").ap()

    sig_scale = float(temp_val) * (HD ** -0.5)

    with tile.TileContext(nc) as tc:
        _emit(nc, tc, d_in, z2T_d, connT_d, float(dt_val), sig_scale)
    nc.compile()
    return nc


def _emit(nc, tc, d_in, z2T_d, connT_d, dt_val, sig_scale):
    from contextlib import ExitStack

    ctx = ExitStack()
    with ctx:
        # ---------- persistent pools ----------
        const = ctx.enter_context(tc.tile_pool(name="const", bufs=1))
        persist = ctx.enter_context(tc.tile_pool(name="persist", bufs=1))
        wpool = ctx.enter_context(tc.tile_pool(name="wpool", bufs=4))
        ps_lin = ctx.enter_context(
            tc.tile_pool(name="ps_lin", bufs=2, space="PSUM"))

        # constants: biases, rms weights, ones
        bias = {}
        for name in ["fb1", "fb2", "gb1", "gb2", "cub1", "cub2", "mb1", "mb2"]:
            n = d_in[name].shape[0]
            t = const.tile([128, n // 128], F32, tag=name)
            nc.sync.dma_start(out=t[:], in_=_bias_ap(d_in[name]))
            bias[name] = t
        wrow = {}
        for name in ["wz", "wc", "wmlp"]:
            tf = const.tile([1, D], F32, tag="wstage")
            nc.sync.dma_start(out=tf[:], in_=d_in[name][None, :])
            tb = const.tile([1, D], BF16, tag=name)
            nc.vector.tensor_copy(tb[:], tf[:])
            wrow[name] = tb
        ones_col = const.tile([128, 1], BF16, tag="ones")
        nc.vector.memset(ones_col[:], 1.0)
        eps1 = const.tile([1, 1], F32, tag="eps1")
        nc.vector.memset(eps1[:], EPS)

        # persistent activations (own tokens, feature-major, bf16)
        zn_own = persist.tile([128, KC, TOK], BF16, tag="zn_own")
        cn_own = persist.tile([128, KC, TOK], BF16, tag="cn_own")
        attnT = persist.tile([64, H, TOK], BF16, tag="attnT")

        # ---------- generic feature-major linear ----------
        def linear_fm(wT_d, n_in, n_out, rhs_fn, evict_fn, wtag="w"):
            # out[m-chunk] = sum_k wT[k,m].T @ rhs(k); evict_fn(mc, psum_ap)
            nob = n_out // OBW
            kcn = n_in // 128
            for ob in range(nob):
                ps = ps_lin.tile([128, 2, 512], F32, tag="lin")
                for k in range(kcn):
                    w = wpool.tile([128, OBW], BF16, tag=wtag)
                    nc.sync.dma_start(
                        out=w[:],
                        in_=wT_d[k * 128:(k + 1) * 128, ob * OBW:(ob + 1) * OBW])
                    for m in range(2):
                        nc.tensor.matmul(
                            ps[:, m, :TOK], w[:, m * 128:(m + 1) * 128],
                            rhs_fn(k), start=(k == 0), stop=(k == kcn - 1))
                for m in range(2):
                    evict_fn(ob * 2 + m, ps[:, m, :TOK])

        # ---------- phase 1+2: norms, K, V, Q ----------
        with tc.tile_pool(name="kvq", bufs=1) as kvq:
            KT = kvq.tile([128, KC, KEYS], BF16, tag="KT")
            V_sb = kvq.tile([128, H, H, HD + 1], BF16, tag="V")
            QT_z = kvq.tile([128, H, TOK], BF16, tag="QT")
            nc.vector.memset(QT_z[:], 0.0)
            nc.vector.memset(V_sb[:, :, :, HD:HD + 1], 1.0)
            norm_scope = ExitStack()
            nrm = norm_scope.enter_context(tc.tile_pool(name="nrm", bufs=1))
            xrawp = norm_scope.enter_context(
                tc.tile_pool(name="xraw", bufs=3))
            ps_nrm = norm_scope.enter_context(
                tc.tile_pool(name="ps_nrm", bufs=1, space="PSUM"))
            ps_ss = norm_scope.enter_context(
                tc.tile_pool(name="ps_ss", bufs=2, space="PSUM"))

            def norm_block(xT_d, w_t, dst, raw_dst=None):
                # one token-block norm: returns nothing; writes normed bf16
                # chunks into dst [128, KC, TOK]
                ss = ps_ss.tile([1, TOK], F32, tag="ss")
                xb = nrm.tile([128, KC, TOK], BF16, tag="xbf")
                for k in range(KC):
                    xf = xrawp.tile([128, TOK], F32, tag="xf")
                    nc.sync.dma_start(out=xf[:], in_=xT_d[k])
                    nc.vector.tensor_copy(xb[:, k, :], xf[:])
                    sq = xrawp.tile([128, TOK], BF16, tag="sq", bufs=2)
                    nc.vector.tensor_mul(sq[:], xb[:, k, :], xb[:, k, :])
                    nc.tensor.matmul(ss[:], ones_col[:], sq[:],
                                     start=(k == 0), stop=(k == KC - 1))
                sf = xrawp.tile([1, TOK], F32, tag="sf", bufs=1)
                nc.scalar.activation(sf[:], ss[:], AF.Sqrt,
                                     bias=eps1[:], scale=1.0 / D)
                nc.vector.reciprocal(sf[:], sf[:])
                sb = xrawp.tile([1, TOK], BF16, tag="sb", bufs=1)
                nc.vector.tensor_copy(sb[:], sf[:])
                for k in range(KC):
                    bc = ps_nrm.tile([128, TOK], F32, tag="bc")
                    nc.tensor.matmul(bc[:], w_t[:, k * 128:(k + 1) * 128],
                                     sb[:], start=True, stop=True)
                    nc.vector.tensor_mul(dst[:, k, :], xb[:, k, :], bc[:])

            for tb in range(NTB):
                cols = slice(tb * TOK, (tb + 1) * TOK)
                zslices = [d_in["zT"][k * 128:(k + 1) * 128, cols]
                           for k in range(KC)]
                cslices = [d_in["cT"][k * 128:(k + 1) * 128, cols]
                           for k in range(KC)]
                if tb == 0:
                    zn_tb, cn_tb = zn_own, cn_own
                    norm_block(zslices, wrow["wz"], zn_tb)
                    norm_block(cslices, wrow["wc"], cn_tb)
                else:
                    zn_tb = nrm.tile([128, KC, TOK], BF16, tag="zn_tb",
                                     bufs=2)
                    cn_tb = nrm.tile([128, KC, TOK], BF16, tag="cn_tb")
                    norm_block(zslices, wrow["wz"], zn_tb)
                    norm_block(cslices, wrow["wc"], cn_tb)

                # K for this token block -> KT[:, :, tb]
                def k_rhs(k):
                    return (zn_tb[:, k, :] if k < KC
                            else cn_tb[:, k - KC, :])

                def k_evict(mc, ps):
                    nc.scalar.activation(KT[:, mc, cols], ps, AF.Copy)

                linear_fm(d_in["kwT"], 2 * D, D, k_rhs, k_evict, wtag="kw")

                # V for this token block (token-major with ones column)
                for kc4 in range(4):
                    kcg = tb * 4 + kc4
                    ps = ps_lin.tile([128, 2, 512], F32, tag="lin")
                    for k in range(KC):
                        lhs = zn_tb[:, k, kc4 * 128:(kc4 + 1) * 128]
                        for vb in range(2):
                            vw = wpool.tile([128, 512], BF16, tag="vw",
                                            bufs=3)
                            nc.sync.dma_start(
                                out=vw[:],
                                in_=d_in["vwT"][k * 128:(k + 1) * 128,
                                                vb * 512:(vb + 1) * 512])
                            nc.tensor.matmul(
                                ps[:, vb, :], lhs, vw[:],
                                start=(k == 0), stop=(k == KC - 1))
                    for vb in range(2):
                        src = ps[:, vb, :].rearrange("p (h d) -> p h d", h=8)
                        nc.scalar.activation(
                            V_sb[:, kcg, vb * 8:(vb + 1) * 8, 0:HD], src,
                            AF.Copy)

                if tb == 0:
                    # Q projection (own tokens), zero-padded per head
                    def q_rhs(k):
                        return (zn_own[:, k, :] if k < KC
                                else cn_own[:, k - KC, :])

                    def q_evict(mc, ps):
                        nc.scalar.activation(
                            QT_z[0:64, 2 * mc, :], ps[0:64, :], AF.Copy)
                        nc.scalar.activation(
                            QT_z[64:128, 2 * mc + 1, :], ps[64:128, :],
                            AF.Copy)

                    linear_fm(d_in["qwT"], 2 * D, D, q_rhs, q_evict,
                              wtag="qw")

            norm_scope.close()

            # ---------- phase 3: sigmoid attention ----------
            with (
                tc.tile_pool(name="rel", bufs=1) as relp,
                tc.tile_pool(name="att_s", bufs=2) as attsp,
                tc.tile_pool(name="ps_sc", bufs=2, space="PSUM") as ps_sc,
                tc.tile_pool(name="ps_av", bufs=2, space="PSUM") as ps_av,
            ):
                for h in range(H):
                    rel = relp.tile([128, H, TOK], BF16, tag="rel")
                    for kc in range(H):
                        sc = ps_sc.tile([128, TOK], F32, tag="sc")
                        nc.tensor.matmul(
                            sc[:], KT[:, h // 2, kc * 128:(kc + 1) * 128],
                            QT_z[:, h, :], start=True, stop=True)
                        nc.scalar.activation(rel[:, kc, :], sc[:], AF.Sigmoid,
                                             scale=sig_scale)
                    av = ps_av.tile([65, TOK], F32, tag="av")
                    for kc in range(H):
                        nc.tensor.matmul(av[:], V_sb[:, kc, h, :],
                                         rel[:, kc, :],
                                         start=(kc == 0), stop=(kc == H - 1))
                    # rel_sum = clip(row 64, 1, inf); recip; bcast; divide
                    rs = attsp.tile([1, TOK], F32, tag="rs")
                    nc.vector.tensor_scalar_max(rs[0:1, :], av[64:65, :], 1.0)
                    nc.vector.reciprocal(rs[0:1, :], rs[0:1, :])
                    bcv = attsp.tile([64, TOK], F32, tag="bcv")
                    nc.gpsimd.partition_broadcast(bcv[:], rs[0:1, :])
                    nc.vector.tensor_mul(attnT[0:64, h, :], av[0:64, :],
                                         bcv[:])

        # ---------- phase 4: dz MLPs, o-proj, cu, final MLP ----------
        with (
            tc.tile_pool(name="mlp", bufs=1) as mlp,
            tc.tile_pool(name="outp", bufs=2) as outp,
            tc.tile_pool(name="ps_ss2", bufs=2, space="PSUM") as ps_ss2,
            tc.tile_pool(name="ps_nrm2", bufs=1, space="PSUM") as ps_nrm2,
        ):
            def evict_silu(dst, ps, bias_ap):
                # silu(x) = x * sigmoid(x); CoreSim has no native Silu
                sg = mlp.tile([128, TOK], BF16, tag="sg", bufs=3)
                nc.scalar.activation(sg[:], ps, AF.Sigmoid, bias=bias_ap)
                nc.vector.scalar_tensor_tensor(
                    dst, ps, bias_ap, sg[:],
                    op0=mybir.AluOpType.add, op1=mybir.AluOpType.mult)

            # hid: fh(16) -> du(16) -> mh(32) share one 32KB slot via tag
            fh = mlp.tile([128, 32, TOK], BF16, tag="hid")
            gh = mlp.tile([128, KC, TOK], BF16, tag="mid8")
            dzl_b = mlp.tile([128, KC, TOK], BF16, tag="dzl")
            s_b = mlp.tile([128, KC, TOK], BF16, tag="s_b")
            s_f = mlp.tile([128, KC, TOK], F32, tag="s_f")

            # f MLP: fh = silu(zn @ f_w1.T + fb1)
            def f1_evict(mc, ps):
                evict_silu(fh[:, mc, :], ps, bias["fb1"][:, mc:mc + 1])

            linear_fm(d_in["fw1T"], D, 2 * D,
                      lambda k: zn_own[:, k, :], f1_evict)

            # dzl = fh @ f_w2.T + fb2 (kept bf16 only)
            def f2_evict(mc, ps):
                nc.vector.tensor_scalar_add(dzl_b[:, mc, :], ps,
                                            bias["fb2"][:, mc:mc + 1])

            linear_fm(d_in["fw2T"], 2 * D, D,
                      lambda k: fh[:, k, :], f2_evict)

            # gh = tanh(cat(cn, dzl) @ g_w1.T + gb1)
            def g1_evict(mc, ps):
                nc.scalar.activation(gh[:, mc, :], ps, AF.Tanh,
                                     bias=bias["gb1"][:, mc:mc + 1])

            linear_fm(d_in["gw1T"], 2 * D, D,
                      lambda k: cn_own[:, k, :] if k < KC
                      else dzl_b[:, k - KC, :], g1_evict)

            # s = dzl + (gh @ g_w2.T + gb2)   (dz = dt*s)
            def g2_evict(mc, ps):
                nc.vector.scalar_tensor_tensor(
                    s_f[:, mc, :], ps, bias["gb2"][:, mc:mc + 1],
                    dzl_b[:, mc, :], op0=mybir.AluOpType.add,
                    op1=mybir.AluOpType.add)
                nc.vector.tensor_copy(s_b[:, mc, :], s_f[:, mc, :])

            linear_fm(d_in["gw2T"], D, D, lambda k: gh[:, k, :], g2_evict)

            # ctx = attn @ o_w.T ; z1 = z + dt*s + ctx
            z1_f = mlp.tile([128, KC, TOK], F32, tag="z1f")
            z1_b = mlp.tile([128, KC, TOK], BF16, tag="z1b")

            nob = D // OBW
            for ob in range(nob):
                ps = ps_lin.tile([128, 2, 512], F32, tag="lin")
                for h in range(H):
                    w = mlp.tile([64, OBW], BF16, tag="ow", bufs=4)
                    nc.sync.dma_start(
                        out=w[:],
                        in_=d_in["owT"][h, :, ob * OBW:(ob + 1) * OBW])
                    for m in range(2):
                        nc.tensor.matmul(
                            ps[:, m, :TOK], w[:, m * 128:(m + 1) * 128],
                            attnT[0:64, h, :], start=(h == 0),
                            stop=(h == H - 1))
                for m in range(2):
                    mc = ob * 2 + m
                    zot = mlp.tile([128, TOK], F32, tag="zot", bufs=2)
                    nc.sync.dma_start(
                        out=zot[:],
                        in_=d_in["zT"][mc * 128:(mc + 1) * 128, 0:TOK])
                    t = mlp.tile([128, TOK], F32, tag="t_z1", bufs=2)
                    nc.vector.scalar_tensor_tensor(
                        t[:], s_f[:, mc, :], dt_val, ps[:, m, :TOK],
                        op0=mybir.AluOpType.mult, op1=mybir.AluOpType.add)
                    nc.vector.tensor_add(z1_f[:, mc, :], t[:], zot[:])
                    nc.vector.tensor_copy(z1_b[:, mc, :], z1_f[:, mc, :])

            # cu: du = silu(cat(c, z1, dt*s) @ cu_w1.T + cub1)
            c_raw = mlp.tile([128, KC, TOK], BF16, tag="c_raw")
            for k in range(KC):
                ct = mlp.tile([128, TOK], F32, tag="zot", bufs=2)
                nc.sync.dma_start(
                    out=ct[:], in_=d_in["cT"][k * 128:(k + 1) * 128, 0:TOK])
                nc.vector.tensor_copy(c_raw[:, k, :], ct[:])

            du = mlp.tile([128, 32, TOK], BF16, tag="hid")

            def cu1_rhs(k):
                if k < KC:
                    return c_raw[:, k, :]
                if k < 2 * KC:
                    return z1_b[:, k - KC, :]
                return s_b[:, k - 2 * KC, :]

            def cu1_evict(mc, ps):
                evict_silu(du[:, mc, :], ps, bias["cub1"][:, mc:mc + 1])

            linear_fm(d_in["cuw1T"], 3 * D, 2 * D, cu1_rhs, cu1_evict)

            # conn_new = c + (du @ cu_w2.T + cub2)
            def cu2_evict(mc, ps):
                ct = mlp.tile([128, TOK], F32, tag="zot", bufs=2)
                nc.sync.dma_start(
                    out=ct[:], in_=d_in["cT"][mc * 128:(mc + 1) * 128, 0:TOK])
                co = outp.tile([128, TOK], F32, tag="co")
                nc.vector.scalar_tensor_tensor(
                    co[:], ps, bias["cub2"][:, mc:mc + 1], ct[:],
                    op0=mybir.AluOpType.add, op1=mybir.AluOpType.add)
                nc.sync.dma_start(
                    out=connT_d[mc * 128:(mc + 1) * 128, :], in_=co[:])

            linear_fm(d_in["cuw2T"], 2 * D, D,
                      lambda k: du[:, k, :], cu2_evict)

            # z1n = rms(z1) * wmlp
            z1n = mlp.tile([128, KC, TOK], BF16, tag="mid8")
            ss = ps_ss2.tile([1, TOK], F32, tag="ss2")
            for k in range(KC):
                sq = mlp.tile([128, TOK], BF16, tag="sq2", bufs=2)
                nc.vector.tensor_mul(sq[:], z1_b[:, k, :], z1_b[:, k, :])
                nc.tensor.matmul(ss[:], ones_col[:], sq[:],
                                 start=(k == 0), stop=(k == KC - 1))
            sf = mlp.tile([1, TOK], F32, tag="sf2")
            nc.scalar.activation(sf[:], ss[:], AF.Sqrt, bias=eps1[:],
                                 scale=1.0 / D)
            nc.vector.reciprocal(sf[:], sf[:])
            sb2 = mlp.tile([1, TOK], BF16, tag="sb2")
            nc.vector.tensor_copy(sb2[:], sf[:])
            for k in range(KC):
                bc = ps_nrm2.tile([128, TOK], F32, tag="bc2")
                nc.tensor.matmul(bc[:], wrow["wmlp"][:, k * 128:(k + 1) * 128],
                                 sb2[:], start=True, stop=True)
                nc.vector.tensor_mul(z1n[:, k, :], z1_b[:, k, :], bc[:])

            # mh = silu(z1n @ m_w1.T + mb1)
            mh = mlp.tile([128, 32, TOK], BF16, tag="hid")

            def m1_evict(mc, ps):
                evict_silu(mh[:, mc, :], ps, bias["mb1"][:, mc:mc + 1])

            linear_fm(d_in["mw1T"], D, 4 * D,
                      lambda k: z1n[:, k, :], m1_evict)

            # z2 = z1 + (mh @ m_w2.T + mb2)
            def m2_evict(mc, ps):
                zo = outp.tile([128, TOK], F32, tag="zo")
                nc.vector.scalar_tensor_tensor(
                    zo[:], ps, bias["mb2"][:, mc:mc + 1], z1_f[:, mc, :],
                    op0=mybir.AluOpType.add, op1=mybir.AluOpType.add)
                nc.sync.dma_start(
                    out=z2T_d[mc * 128:(mc + 1) * 128, :], in_=zo[:])

            linear_fm(d_in["mw2T"], 4 * D, D,
                      lambda k: mh[:, k, :], m2_evict)


_CACHE = {}


def _prep_shared(inputs):
    bf = ml_dtypes.bfloat16

    def t(x, dt=bf):
        return np.ascontiguousarray(np.asarray(x, np.float32).T).astype(dt)

    dt_val = float(np.asarray(inputs["dt"]))
    cu1 = np.asarray(inputs["cu_w1"], np.float32).copy()
    cu1[:, 2 * D:] *= dt_val  # fold dz = dt*s into cu_w1's dz block
    shared = {
        "fw1T": t(inputs["f_w1"]), "fw2T": t(inputs["f_w2"]),
        "gw1T": t(inputs["g_w1"]), "gw2T": t(inputs["g_w2"]),
        "qwT": t(inputs["q_w"]), "kwT": t(inputs["k_w"]),
        "vwT": t(inputs["v_w"]),
        "owT": np.ascontiguousarray(
            np.asarray(inputs["o_w"], np.float32).T.reshape(H, HD, D)
        ).astype(bf),
        "cuw1T": np.ascontiguousarray(cu1.T).astype(bf),
        "cuw2T": t(inputs["cu_w2"]),
        "mw1T": t(inputs["m_w1"]), "mw2T": t(inputs["m_w2"]),
    }
    for name, key in [("fb1", "f_b1"), ("fb2", "f_b2"), ("gb1", "g_b1"),
                      ("gb2", "g_b2"), ("cub1", "cu_b1"), ("cub2", "cu_b2"),
                      ("mb1", "m_b1"), ("mb2", "m_b2"), ("wz", "w_z"),
                      ("wc", "w_c"), ("wmlp", "w_mlp")]:
        shared[name] = np.ascontiguousarray(np.asarray(inputs[key], np.float32))
    return shared


def kernel(**inputs):
    z = np.asarray(inputs["z"], np.float32)
    conn = np.asarray(inputs["connection"], np.float32)
    dt_val = float(np.asarray(inputs["dt"]))
    temp_val = float(np.asarray(inputs["temp"]))

    key = (dt_val, temp_val)
    if key not in _CACHE:
        _CACHE[key] = build_program(dt_val, temp_val)
    nc = _CACHE[key]

    shared = _prep_shared(inputs)
    zT = [np.ascontiguousarray(z[b].T) for b in range(B)]
    cT = [np.ascontiguousarray(conn[b].T) for b in range(B)]

    in_maps = []
    for c in range(NCORES):
        b, tb = divmod(c, NTB)
        m = dict(shared)
        m["zT"] = np.ascontiguousarray(np.roll(zT[b], -tb * TOK, axis=1))
        m["cT"] = np.ascontiguousarray(np.roll(cT[b], -tb * TOK, axis=1))
        in_maps.append(m)

    res = run_bass_kernel_spmd(nc, in_maps, list(range(NCORES)))

    z2 = np.empty((B, L, D), np.float32)
    conn_new = np.empty((B, L, D), np.float32)
    for c in range(NCORES):
        b, tb = divmod(c, NTB)
        sl = slice(tb * TOK, (tb + 1) * TOK)
        z2[b, sl, :] = res.results[c]["z2T"].T
        conn_new[b, sl, :] = res.results[c]["connT"].T
    return z2, conn_new, z


# revision 31
# speedup vs baseline: 1.1033x; 1.1033x over previous
"""CovariantEvolutionBlock Trainium2 kernel.

Strategy: token-parallel over B*L across 8 cores (512 tokens/core), zero
collectives. Each core recomputes full-batch K/V for attention (inputs are
rotated per-core so "own" tokens are always columns 0:512; sigmoid attention
is permutation-invariant over keys). Activations are kept feature-major
[dims, tokens] on-chip so matmul chains need no transposes; weights are
pre-transposed/cast to bf16 on the host. All matmuls are bf16 with fp32 PSUM
accumulation.
"""

import sys

try:
    import concourse.bass as bass  # noqa: F401
except ImportError:
    sys.path.insert(0, "/opt/trn_rl_repo")

import numpy as np
import ml_dtypes

import concourse.bacc as bacc
import concourse.tile as tile
import concourse.mybir as mybir
from concourse.bass_utils import run_bass_kernel_spmd

F32 = mybir.dt.float32
BF16 = mybir.dt.bfloat16
AF = mybir.ActivationFunctionType

B, L, D, H, HD = 2, 2048, 1024, 16, 64
EPS = 1e-6
NCORES = 8
TOK = 512          # own tokens per core
KEYS = 2048        # keys per batch
KC = D // 128      # 8 feature chunks of 128
NTB = KEYS // TOK  # 4 token blocks per batch
OBW = 256          # out-block width (2 m-chunks) per psum tile


def _bias_ap(dram_ap):
    # [dim] -> [128, dim//128]: tile[p, c] = bias[c*128 + p]
    return dram_ap.rearrange("(c p) -> p c", p=128)


def build_program(dt_val: float, temp_val: float):
    nc = bacc.Bacc("TRN2", target_bir_lowering=False, debug=False,
                   num_devices=NCORES)

    d_in = {}
    for name, shape, dt in [
        ("zT", [D, KEYS], F32), ("cT", [D, KEYS], F32),
        ("fw1T", [D, 2 * D], BF16), ("fw2T", [2 * D, D], BF16),
        ("gw1T", [2 * D, D], BF16), ("gw2T", [D, D], BF16),
        ("qwT", [2 * D, D], BF16), ("kwT", [2 * D, D], BF16),
        ("vwT", [D, D], BF16), ("owT", [D, D], BF16),
        ("cuw1T", [3 * D, 2 * D], BF16), ("cuw2T", [2 * D, D], BF16),
        ("mw1T", [D, 4 * D], BF16), ("mw2T", [4 * D, D], BF16),
        ("fb1", [2 * D], F32), ("fb2", [D], F32),
        ("gb1", [D], F32), ("gb2", [D], F32),
        ("cub1", [2 * D], F32), ("cub2", [D], F32),
        ("mb1", [4 * D], F32), ("mb2", [D], F32),
        ("wz", [D], F32), ("wc", [D], F32), ("wmlp", [D], F32),
    ]:
        d_in[name] = nc.dram_tensor(name, shape, dt, kind="ExternalInput").ap()

    z2T_d = nc.dram_tensor("z2T", [D, TOK], F32, kind="ExternalOutput").ap()
    connT_d = nc.dram_tensor("connT", [D, TOK], F32, kind="ExternalOutput").ap()

    sig_scale = float(temp_val) * (HD ** -0.5)

    with tile.TileContext(nc) as tc:
        _emit(nc, tc, d_in, z2T_d, connT_d, float(dt_val), sig_scale)
    nc.compile()
    return nc


def _emit(nc, tc, d_in, z2T_d, connT_d, dt_val, sig_scale):
    from contextlib import ExitStack

    ctx = ExitStack()
    with ctx:
        # ---------- persistent pools ----------
        const = ctx.enter_context(tc.tile_pool(name="const", bufs=1))
        persist = ctx.enter_context(tc.tile_pool(name="persist", bufs=1))
        wpool = ctx.enter_context(tc.tile_pool(name="wpool", bufs=4))
        ps_lin = ctx.enter_context(
            tc.tile_pool(name="ps_lin", bufs=2, space="PSUM"))

        # constants: biases, rms weights, ones
        bias = {}
        for name in ["fb1", "fb2", "gb1", "gb2", "cub1", "cub2", "mb1", "mb2"]:
            n = d_in[name].shape[0]
            t = const.tile([128, n // 128], F32, tag=name)
            nc.sync.dma_start(out=t[:], in_=_bias_ap(d_in[name]))
            bias[name] = t
        wcol = {}
        for name in ["wz", "wc", "wmlp"]:
            t = const.tile([128, KC], F32, tag=name)
            nc.sync.dma_start(out=t[:], in_=_bias_ap(d_in[name]))
            wcol[name] = t
        ones_col = const.tile([128, 1], BF16, tag="ones")
        nc.vector.memset(ones_col[:], 1.0)
        eps1 = const.tile([1, 1], F32, tag="eps1")
        nc.vector.memset(eps1[:], EPS)

        # persistent activations (own tokens, feature-major, bf16)
        zn_own = persist.tile([128, KC, TOK], BF16, tag="zn_own")
        cn_own = persist.tile([128, KC, TOK], BF16, tag="cn_own")
        attnT = persist.tile([128, KC, TOK], BF16, tag="attnT")

        # ---------- generic feature-major linear ----------
        def linear_fm(wT_d, n_in, n_out, rhs_fn, evict_fn, wtag="w"):
            # out[m-chunk] = sum_k wT[k,m].T @ rhs(k); evict_fn(mc, psum_ap)
            nob = n_out // OBW
            kcn = n_in // 128
            for ob in range(nob):
                ps = ps_lin.tile([128, 2, 512], F32, tag="lin")
                for k in range(kcn):
                    w = wpool.tile([128, OBW], BF16, tag=wtag, bufs=10)
                    nc.sync.dma_start(
                        out=w[:],
                        in_=wT_d[k * 128:(k + 1) * 128, ob * OBW:(ob + 1) * OBW])
                    for m in range(2):
                        nc.tensor.matmul(
                            ps[:, m, :TOK], w[:, m * 128:(m + 1) * 128],
                            rhs_fn(k), start=(k == 0), stop=(k == kcn - 1))
                for m in range(2):
                    evict_fn(ob * 2 + m, ps[:, m, :TOK])

        # ---------- phase 1+2: norms, K, V, Q ----------
        with tc.tile_pool(name="kvq", bufs=1) as kvq:
            KT = kvq.tile([128, KC, KEYS], BF16, tag="KT")
            V_sb = kvq.tile([128, H, H, HD + 1], BF16, tag="V")
            QT_z = kvq.tile([128, H, TOK], BF16, tag="QT")
            nc.vector.memset(QT_z[:], 0.0)
            nc.vector.memset(V_sb[:, :, :, HD:HD + 1], 1.0)
            norm_scope = ExitStack()
            nrm = norm_scope.enter_context(tc.tile_pool(name="nrm", bufs=1))
            xrawp = norm_scope.enter_context(
                tc.tile_pool(name="xraw", bufs=3))
            ps_ss = norm_scope.enter_context(
                tc.tile_pool(name="ps_ss", bufs=2, space="PSUM"))

            def norm_block(xT_d, w_t, dst, raw_dst=None):
                # one token-block norm: returns nothing; writes normed bf16
                # chunks into dst [128, KC, TOK]
                ss = ps_ss.tile([1, TOK], F32, tag="ss")
                xb = nrm.tile([128, KC, TOK], BF16, tag="xbf", bufs=2)
                for k in range(KC):
                    xf = xrawp.tile([128, TOK], F32, tag="xf")
                    nc.sync.dma_start(out=xf[:], in_=xT_d[k])
                    nc.vector.tensor_copy(xb[:, k, :], xf[:])
                    sq = xrawp.tile([128, TOK], BF16, tag="sq", bufs=3)
                    nc.vector.tensor_mul(sq[:], xb[:, k, :], xb[:, k, :])
                    nc.tensor.matmul(ss[:], ones_col[:], sq[:],
                                     start=(k == 0), stop=(k == KC - 1))
                sf = xrawp.tile([1, TOK], F32, tag="sf", bufs=1)
                nc.scalar.activation(sf[:], ss[:], AF.Sqrt,
                                     bias=eps1[:], scale=1.0 / D)
                nc.vector.reciprocal(sf[:], sf[:])
                bc = xrawp.tile([128, TOK], F32, tag="bc", bufs=2)
                nc.gpsimd.partition_broadcast(bc[:], sf[0:1, :])
                for k in range(KC):
                    nc.vector.scalar_tensor_tensor(
                        dst[:, k, :], bc[:], w_t[:, k:k + 1], xb[:, k, :],
                        op0=mybir.AluOpType.mult, op1=mybir.AluOpType.mult)

            for tb in range(NTB):
                cols = slice(tb * TOK, (tb + 1) * TOK)
                zslices = [d_in["zT"][k * 128:(k + 1) * 128, cols]
                           for k in range(KC)]
                cslices = [d_in["cT"][k * 128:(k + 1) * 128, cols]
                           for k in range(KC)]
                if tb == 0:
                    zn_tb, cn_tb = zn_own, cn_own
                    norm_block(zslices, wcol["wz"], zn_tb)
                    norm_block(cslices, wcol["wc"], cn_tb)
                else:
                    zn_tb = nrm.tile([128, KC, TOK], BF16, tag="zn_tb",
                                     bufs=2)
                    cn_tb = nrm.tile([128, KC, TOK], BF16, tag="cn_tb")
                    norm_block(zslices, wcol["wz"], zn_tb)
                    norm_block(cslices, wcol["wc"], cn_tb)

                # K for this token block -> KT[:, :, tb]
                def k_rhs(k):
                    return (zn_tb[:, k, :] if k < KC
                            else cn_tb[:, k - KC, :])

                def k_evict(mc, ps):
                    nc.scalar.activation(KT[:, mc, cols], ps, AF.Copy)

                linear_fm(d_in["kwT"], 2 * D, D, k_rhs, k_evict, wtag="kw")

                # V for this token block (token-major with ones column)
                for kc4 in range(4):
                    kcg = tb * 4 + kc4
                    ps = ps_lin.tile([128, 2, 512], F32, tag="lin")
                    for k in range(KC):
                        lhs = zn_tb[:, k, kc4 * 128:(kc4 + 1) * 128]
                        for vb in range(2):
                            vw = wpool.tile([128, 512], BF16, tag="vw",
                                            bufs=3)
                            nc.sync.dma_start(
                                out=vw[:],
                                in_=d_in["vwT"][k * 128:(k + 1) * 128,
                                                vb * 512:(vb + 1) * 512])
                            nc.tensor.matmul(
                                ps[:, vb, :], lhs, vw[:],
                                start=(k == 0), stop=(k == KC - 1))
                    for vb in range(2):
                        src = ps[:, vb, :].rearrange("p (h d) -> p h d", h=8)
                        nc.scalar.activation(
                            V_sb[:, kcg, vb * 8:(vb + 1) * 8, 0:HD], src,
                            AF.Copy)

                if tb == 0:
                    # Q projection (own tokens), zero-padded per head
                    def q_rhs(k):
                        return (zn_own[:, k, :] if k < KC
                                else cn_own[:, k - KC, :])

                    def q_evict(mc, ps):
                        nc.scalar.activation(
                            QT_z[0:64, 2 * mc, :], ps[0:64, :], AF.Copy)
                        nc.scalar.activation(
                            QT_z[64:128, 2 * mc + 1, :], ps[64:128, :],
                            AF.Copy)

                    linear_fm(d_in["qwT"], 2 * D, D, q_rhs, q_evict,
                              wtag="qw")

            norm_scope.close()

            # ---------- phase 3: sigmoid attention ----------
            with (
                tc.tile_pool(name="rel", bufs=1) as relp,
                tc.tile_pool(name="att_s", bufs=2) as attsp,
                tc.tile_pool(name="ps_sc", bufs=2, space="PSUM") as ps_sc,
                tc.tile_pool(name="ps_av", bufs=2, space="PSUM") as ps_av,
            ):
                for h in range(H):
                    rel = relp.tile([128, H, TOK], BF16, tag="rel")
                    for kc in range(H):
                        sc = ps_sc.tile([128, TOK], F32, tag="sc")
                        nc.tensor.matmul(
                            sc[:], KT[:, h // 2, kc * 128:(kc + 1) * 128],
                            QT_z[:, h, :], start=True, stop=True)
                        nc.scalar.activation(rel[:, kc, :], sc[:], AF.Sigmoid,
                                             scale=sig_scale)
                    av = ps_av.tile([65, TOK], F32, tag="av")
                    for kc in range(H):
                        nc.tensor.matmul(av[:], V_sb[:, kc, h, :],
                                         rel[:, kc, :],
                                         start=(kc == 0), stop=(kc == H - 1))
                    # rel_sum = clip(row 64, 1, inf); recip; bcast; divide
                    rs = attsp.tile([1, TOK], F32, tag="rs")
                    nc.vector.tensor_scalar_max(rs[0:1, :], av[64:65, :], 1.0)
                    nc.vector.reciprocal(rs[0:1, :], rs[0:1, :])
                    bcv = attsp.tile([64, TOK], F32, tag="bcv")
                    nc.gpsimd.partition_broadcast(bcv[:], rs[0:1, :])
                    po = (h % 2) * 64
                    nc.vector.tensor_mul(attnT[po:po + 64, h // 2, :],
                                         av[0:64, :], bcv[:])

        # ---------- phase 4: dz MLPs, o-proj, cu, final MLP ----------
        with (
            tc.tile_pool(name="mlp", bufs=1) as mlp,
            tc.tile_pool(name="outp", bufs=2) as outp,
            tc.tile_pool(name="ps_ss2", bufs=2, space="PSUM") as ps_ss2,
        ):
            def evict_silu(dst, ps, bias_ap):
                # silu(x) = x * sigmoid(x); CoreSim has no native Silu
                sg = mlp.tile([128, TOK], BF16, tag="sg", bufs=3)
                nc.scalar.activation(sg[:], ps, AF.Sigmoid, bias=bias_ap)
                nc.vector.scalar_tensor_tensor(
                    dst, ps, bias_ap, sg[:],
                    op0=mybir.AluOpType.add, op1=mybir.AluOpType.mult)

            # hid: fh(16) -> du(16) -> mh(32) share one 32KB slot via tag
            fh = mlp.tile([128, 32, TOK], BF16, tag="hid")
            gh = mlp.tile([128, KC, TOK], BF16, tag="mid8")
            dzl_b = mlp.tile([128, KC, TOK], BF16, tag="dzl")
            s_b = mlp.tile([128, KC, TOK], BF16, tag="s_b")
            s_f = mlp.tile([128, KC, TOK], F32, tag="s_f")

            # f MLP: fh = silu(zn @ f_w1.T + fb1)
            def f1_evict(mc, ps):
                evict_silu(fh[:, mc, :], ps, bias["fb1"][:, mc:mc + 1])

            linear_fm(d_in["fw1T"], D, 2 * D,
                      lambda k: zn_own[:, k, :], f1_evict)

            # dzl = fh @ f_w2.T + fb2 (kept bf16 only)
            def f2_evict(mc, ps):
                nc.vector.tensor_scalar_add(dzl_b[:, mc, :], ps,
                                            bias["fb2"][:, mc:mc + 1])

            linear_fm(d_in["fw2T"], 2 * D, D,
                      lambda k: fh[:, k, :], f2_evict)

            # gh = tanh(cat(cn, dzl) @ g_w1.T + gb1)
            def g1_evict(mc, ps):
                nc.scalar.activation(gh[:, mc, :], ps, AF.Tanh,
                                     bias=bias["gb1"][:, mc:mc + 1])

            linear_fm(d_in["gw1T"], 2 * D, D,
                      lambda k: cn_own[:, k, :] if k < KC
                      else dzl_b[:, k - KC, :], g1_evict)

            # s = dzl + (gh @ g_w2.T + gb2)   (dz = dt*s)
            def g2_evict(mc, ps):
                nc.vector.scalar_tensor_tensor(
                    s_f[:, mc, :], ps, bias["gb2"][:, mc:mc + 1],
                    dzl_b[:, mc, :], op0=mybir.AluOpType.add,
                    op1=mybir.AluOpType.add)
                nc.vector.tensor_copy(s_b[:, mc, :], s_f[:, mc, :])

            linear_fm(d_in["gw2T"], D, D, lambda k: gh[:, k, :], g2_evict)

            # ctx = attn @ o_w.T ; z1 = z + dt*s + ctx
            z1_f = mlp.tile([128, KC, TOK], F32, tag="z1f")
            z1_b = mlp.tile([128, KC, TOK], BF16, tag="z1b")

            def o_evict(mc, ps):
                zot = mlp.tile([128, TOK], F32, tag="zot", bufs=2)
                nc.sync.dma_start(
                    out=zot[:],
                    in_=d_in["zT"][mc * 128:(mc + 1) * 128, 0:TOK])
                t = mlp.tile([128, TOK], F32, tag="t_z1", bufs=2)
                nc.vector.scalar_tensor_tensor(
                    t[:], s_f[:, mc, :], dt_val, ps,
                    op0=mybir.AluOpType.mult, op1=mybir.AluOpType.add)
                nc.vector.tensor_add(z1_f[:, mc, :], t[:], zot[:])
                nc.vector.tensor_copy(z1_b[:, mc, :], z1_f[:, mc, :])

            linear_fm(d_in["owT"], D, D,
                      lambda k: attnT[:, k, :], o_evict)

            # cu: du = silu(cat(c, z1, dt*s) @ cu_w1.T + cub1)
            c_raw = mlp.tile([128, KC, TOK], BF16, tag="c_raw")
            for k in range(KC):
                ct = mlp.tile([128, TOK], F32, tag="zot", bufs=2)
                nc.sync.dma_start(
                    out=ct[:], in_=d_in["cT"][k * 128:(k + 1) * 128, 0:TOK])
                nc.vector.tensor_copy(c_raw[:, k, :], ct[:])

            du = mlp.tile([128, 32, TOK], BF16, tag="hid")

            def cu1_rhs(k):
                if k < KC:
                    return c_raw[:, k, :]
                if k < 2 * KC:
                    return z1_b[:, k - KC, :]
                return s_b[:, k - 2 * KC, :]

            def cu1_evict(mc, ps):
                evict_silu(du[:, mc, :], ps, bias["cub1"][:, mc:mc + 1])

            linear_fm(d_in["cuw1T"], 3 * D, 2 * D, cu1_rhs, cu1_evict)

            # conn_new = c + (du @ cu_w2.T + cub2)
            def cu2_evict(mc, ps):
                ct = mlp.tile([128, TOK], F32, tag="zot", bufs=2)
                nc.sync.dma_start(
                    out=ct[:], in_=d_in["cT"][mc * 128:(mc + 1) * 128, 0:TOK])
                co = outp.tile([128, TOK], F32, tag="co")
                nc.vector.scalar_tensor_tensor(
                    co[:], ps, bias["cub2"][:, mc:mc + 1], ct[:],
                    op0=mybir.AluOpType.add, op1=mybir.AluOpType.add)
                nc.sync.dma_start(
                    out=connT_d[mc * 128:(mc + 1) * 128, :], in_=co[:])

            linear_fm(d_in["cuw2T"], 2 * D, D,
                      lambda k: du[:, k, :], cu2_evict)

            # z1n = rms(z1) * wmlp
            z1n = mlp.tile([128, KC, TOK], BF16, tag="mid8")
            ss = ps_ss2.tile([1, TOK], F32, tag="ss2")
            for k in range(KC):
                sq = mlp.tile([128, TOK], BF16, tag="sq2", bufs=2)
                nc.vector.tensor_mul(sq[:], z1_b[:, k, :], z1_b[:, k, :])
                nc.tensor.matmul(ss[:], ones_col[:], sq[:],
                                 start=(k == 0), stop=(k == KC - 1))
            sf = mlp.tile([1, TOK], F32, tag="sf2")
            nc.scalar.activation(sf[:], ss[:], AF.Sqrt, bias=eps1[:],
                                 scale=1.0 / D)
            nc.vector.reciprocal(sf[:], sf[:])
            bc2 = mlp.tile([128, TOK], F32, tag="bc2")
            nc.gpsimd.partition_broadcast(bc2[:], sf[0:1, :])
            for k in range(KC):
                nc.vector.scalar_tensor_tensor(
                    z1n[:, k, :], bc2[:], wcol["wmlp"][:, k:k + 1],
                    z1_b[:, k, :], op0=mybir.AluOpType.mult,
                    op1=mybir.AluOpType.mult)

            # mh = silu(z1n @ m_w1.T + mb1)
            mh = mlp.tile([128, 32, TOK], BF16, tag="hid")

            def m1_evict(mc, ps):
                evict_silu(mh[:, mc, :], ps, bias["mb1"][:, mc:mc + 1])

            linear_fm(d_in["mw1T"], D, 4 * D,
                      lambda k: z1n[:, k, :], m1_evict)

            # z2 = z1 + (mh @ m_w2.T + mb2)
            def m2_evict(mc, ps):
                zo = outp.tile([128, TOK], F32, tag="zo")
                nc.vector.scalar_tensor_tensor(
                    zo[:], ps, bias["mb2"][:, mc:mc + 1], z1_f[:, mc, :],
                    op0=mybir.AluOpType.add, op1=mybir.AluOpType.add)
                nc.sync.dma_start(
                    out=z2T_d[mc * 128:(mc + 1) * 128, :], in_=zo[:])

            linear_fm(d_in["mw2T"], 4 * D, D,
                      lambda k: mh[:, k, :], m2_evict)


_CACHE = {}


def _prep_shared(inputs):
    bf = ml_dtypes.bfloat16

    def t(x, dt=bf):
        return np.ascontiguousarray(np.asarray(x, np.float32).T).astype(dt)

    dt_val = float(np.asarray(inputs["dt"]))
    cu1 = np.asarray(inputs["cu_w1"], np.float32).copy()
    cu1[:, 2 * D:] *= dt_val  # fold dz = dt*s into cu_w1's dz block
    shared = {
        "fw1T": t(inputs["f_w1"]), "fw2T": t(inputs["f_w2"]),
        "gw1T": t(inputs["g_w1"]), "gw2T": t(inputs["g_w2"]),
        "qwT": t(inputs["q_w"]), "kwT": t(inputs["k_w"]),
        "vwT": t(inputs["v_w"]),
        "owT": t(inputs["o_w"]),
        "cuw1T": np.ascontiguousarray(cu1.T).astype(bf),
        "cuw2T": t(inputs["cu_w2"]),
        "mw1T": t(inputs["m_w1"]), "mw2T": t(inputs["m_w2"]),
    }
    for name, key in [("fb1", "f_b1"), ("fb2", "f_b2"), ("gb1", "g_b1"),
                      ("gb2", "g_b2"), ("cub1", "cu_b1"), ("cub2", "cu_b2"),
                      ("mb1", "m_b1"), ("mb2", "m_b2"), ("wz", "w_z"),
                      ("wc", "w_c"), ("wmlp", "w_mlp")]:
        shared[name] = np.ascontiguousarray(np.asarray(inputs[key], np.float32))
    return shared


def kernel(**inputs):
    z = np.asarray(inputs["z"], np.float32)
    conn = np.asarray(inputs["connection"], np.float32)
    dt_val = float(np.asarray(inputs["dt"]))
    temp_val = float(np.asarray(inputs["temp"]))

    key = (dt_val, temp_val)
    if key not in _CACHE:
        _CACHE[key] = build_program(dt_val, temp_val)
    nc = _CACHE[key]

    shared = _prep_shared(inputs)
    zT = [np.ascontiguousarray(z[b].T) for b in range(B)]
    cT = [np.ascontiguousarray(conn[b].T) for b in range(B)]

    in_maps = []
    for c in range(NCORES):
        b, tb = divmod(c, NTB)
        m = dict(shared)
        m["zT"] = np.ascontiguousarray(np.roll(zT[b], -tb * TOK, axis=1))
        m["cT"] = np.ascontiguousarray(np.roll(cT[b], -tb * TOK, axis=1))
        in_maps.append(m)

    res = run_bass_kernel_spmd(nc, in_maps, list(range(NCORES)))

    z2 = np.empty((B, L, D), np.float32)
    conn_new = np.empty((B, L, D), np.float32)
    for c in range(NCORES):
        b, tb = divmod(c, NTB)
        sl = slice(tb * TOK, (tb + 1) * TOK)
        z2[b, sl, :] = res.results[c]["z2T"].T
        conn_new[b, sl, :] = res.results[c]["connT"].T
    return z2, conn_new, z
